# revision 2
# baseline (speedup 1.0000x reference)
"""Concat cost-volume kernel for Trainium2 (8 NeuronCores, SPMD).

Reference semantics (B=2, C=32, H=128, W=240, D=max_disp=48):
  out[b, c,      d, h, w] = left [b, c, h, w]     * (w >= d)   for c in [0, C)
  out[b, C + c,  d, h, w] = right[b, c, h, w - d] * (w >= d)   for c in [0, C)

Pure data movement (~755 MB of f32 output from ~16 MB of input): the kernel
is HBM-write-bandwidth bound. Design (v2, bf16):

* Precision: the correctness gate is rel_err < 2e-2 against the f32
  reference; bf16 rounds inputs once (elementwise rel err <= 2^-8), so the
  whole volume is built and stored as bf16 — HALVING the HBM write traffic —
  and widened back to f32 on the host during unsharding.

* Sharding: channel-parallel. Core k builds the full disparity volume for
  channels [4k, 4k+4) of BOTH halves — all 8 cores run one identical
  program on different channel slices.

* Per-core output layout [B, 2CPC, H, D, W] (disparity moved inward): each
  store descriptor then covers one SBUF partition row's full [D*W] span —
  23 KB contiguous in HBM instead of the 960 B (w >= d staircase) chunks of
  the [D, H, W] layout, which cost ~9% packet overhead (chunk/32+3 beats).
  The host moves D back in front of H during assembly (np view + cast).
  All planes are written in full (zeros included): skipping the w < d wedge
  would fragment descriptors below 512 B and cost more than the ~10% byte
  saving. No reliance on pre-zeroed output buffers.

* Per-core dataflow:
  - left/right slices [2,4,128,240] f32 -> bf16 SBUF via SWDGE cast-loads
    (gpsimd), right into a zero-padded [128, 8*288] tile (48 zero columns in
    front of each plane) so the shifted+masked right plane for disparity d
    is the sliding window cols [48-d : 288-d].
  - staging: per (b, side, c) unit, build [128h, 48d*240w] bf16 in SBUF with
    D-fused DVE ops (broadcast step-0 / sliding step -1 disparity dim).
    bf16 engages the DVE 2x (tensor_tensor) / 4x (copy) perf modes: the
    innermost AP dim is stride-1 two-byte, which is all the mode needs.
    ~70 us/core, fully hidden under the stores.
  - store: one 2.95 MB DMA per unit (16/core), alternating the two HWDGE
    rings (nc.sync / nc.scalar), each descriptor a full 23 KB partition row.

Expected: ~47.2 MB written/core at ~358 GB/s/NC HBM write roofline
=> ~132 us/core (vs 271 us for the f32 staircase v1).
"""

import dataclasses
import sys

import numpy as np

for _p in ("/opt/trn_rl_repo",):
    if _p not in sys.path:
        sys.path.insert(0, _p)

import concourse.bass as bass  # noqa: F401  (kept for interactive debugging)
import concourse.tile as tile
from concourse import bacc, mybir
from concourse.bass_utils import run_bass_kernel_spmd

B, C, H, W = 2, 32, 128, 240
D = 48
N_CORES = 8
CPC = C // N_CORES  # channels per core (per side) = 4
PAD = D  # zero-pad columns = 48
WPAD = W + PAD  # 288
NBC = B * CPC  # input planes per side per core = 8

BEST = dict(gsz=16, bufs=3)

_NC_CACHE = {}


def _build_nc(gsz=16, bufs=3, loop_n=None, skip_stores=False,
              skip_compute=False, dma_engines=("sync", "scalar"),
              copy_engine="vector", mul_engine="vector", store_split=1,
              unit_order="bsc"):
    """Build (and compile) the per-core SPMD program.

    Only gsz/bufs (via BEST) affect the production kernel; the other knobs
    exist for benchmarking variants (loop_n wraps the body in a hardware
    For_i for delta timing, skip_* isolate engines, store_split splits each
    unit's store across rings).
    """
    nc = bacc.Bacc("TRN2", target_bir_lowering=False, debug=False)
    f32 = mybir.dt.float32
    bf16 = mybir.dt.bfloat16
    left_p = nc.declare_dram_parameter("left", [B, CPC, H, W], f32, isOutput=False)
    right_p = nc.declare_dram_parameter("right", [B, CPC, H, W], f32, isOutput=False)
    out_p = nc.declare_dram_parameter(
        "out", [B, 2 * CPC, H, D, W], bf16, isOutput=True
    )

    with tile.TileContext(nc) as tc:
        with (
            tc.tile_pool(name="consts", bufs=1) as consts,
            tc.tile_pool(name="stage", bufs=bufs) as stagep,
        ):
            left_t = consts.tile([H, NBC * W], bf16)
            right_t = consts.tile([H, NBC * WPAD], bf16)
            mask_t = consts.tile([H, WPAD], bf16)

            nc.vector.memset(mask_t[:, 0:PAD], 0.0)
            nc.vector.memset(mask_t[:, PAD:WPAD], 1.0)
            nc.gpsimd.memset(right_t[:, :], 0.0)

            # f32 -> bf16 cast-loads: dtype conversion during DMA is a
            # SWDGE-only feature, so these go on gpsimd.
            nc.gpsimd.dma_start(
                out=left_t[:, :].rearrange("h (k w) -> h k w", w=W),
                in_=left_p[:, :, :, :].rearrange("b c h w -> h (b c) w"),
            )
            nc.gpsimd.dma_start(
                out=right_t[:, :].rearrange("h (k w) -> h k w", w=WPAD)[:, :, PAD:],
                in_=right_p[:, :, :, :].rearrange("b c h w -> h (b c) w"),
            )

            engs = {"sync": nc.sync, "scalar": nc.scalar, "gpsimd": nc.gpsimd,
                    "vector": nc.vector}
            const_st = None
            if skip_compute:
                const_st = consts.tile([H, D * W], bf16, name="const_st")
                nc.vector.memset(const_st[:, :], 0.5)

            def stage_unit(st, side, bc):
                """Fill st ([128h, 48d * 240w]) with the unit's masked volume.

                One DVE op per group of gsz disparities: the source AP's
                middle dim walks the disparity axis via step 0 (left: same
                plane each d) or step -1 (right / mask: window start col
                48-d slides left as d grows).
                """
                st3d = st[:, :].rearrange("h (d w) -> h d w", w=W)
                for g0 in range(0, D, gsz):
                    dst = st3d[:, g0 : g0 + gsz, :]
                    if side == 0:
                        lb = left_t[:, bc * W : (bc + 1) * W]
                        mb = mask_t[:, PAD - g0 : PAD - g0 + W]
                        engs[mul_engine].tensor_mul(
                            dst,
                            dataclasses.replace(
                                lb, ap=[lb.ap[0], [0, gsz], [1, W]]
                            ),
                            dataclasses.replace(
                                mb, ap=[mb.ap[0], [-1, gsz], [1, W]]
                            ),
                        )
                    else:
                        rb = right_t[
                            :, bc * WPAD + PAD - g0 : bc * WPAD + PAD - g0 + W
                        ]
                        engs[copy_engine].tensor_copy(
                            dst,
                            dataclasses.replace(
                                rb, ap=[rb.ap[0], [-1, gsz], [1, W]]
                            ),
                        )

            def store_unit(st, u, b, cc):
                if store_split == 1:
                    eng = engs[dma_engines[u % len(dma_engines)]]
                    eng.dma_start(
                        out=out_p[b, cc].rearrange("h d w -> h (d w)"),
                        in_=st[:, :],
                    )
                else:
                    step = D // store_split
                    for si in range(store_split):
                        eng = engs[
                            dma_engines[(u * store_split + si) % len(dma_engines)]
                        ]
                        eng.dma_start(
                            out=out_p[b, cc, :, si * step : (si + 1) * step, :]
                            .rearrange("h d w -> h (d w)"),
                            in_=st[:, si * step * W : (si + 1) * step * W],
                        )

            def body():
                if unit_order == "bsc":
                    units = range(2 * NBC)
                else:  # interleave sides: l r l r ...
                    units = [
                        (i // 2) + (i % 2) * CPC + (i // (2 * CPC)) * 2 * CPC
                        for i in range(2 * NBC)
                    ]
                for u in units:
                    b = u // (2 * CPC)
                    side = (u % (2 * CPC)) // CPC
                    c = u % CPC
                    bc = b * CPC + c
                    if skip_compute:
                        st = const_st
                    else:
                        st = stagep.tile([H, D * W], bf16, tag="st", name="st")
                        stage_unit(st, side, bc)
                    if not skip_stores:
                        store_unit(st, u, b, side * CPC + c)

            if loop_n is not None:
                with tc.For_i(0, loop_n, 1):
                    body()
            else:
                body()
    nc.compile()
    return nc


def _get_nc():
    if "nc" not in _NC_CACHE:
        _NC_CACHE["nc"] = _build_nc(**BEST)
    return _NC_CACHE["nc"]


def _make_in_maps(left, right):
    in_maps = []
    for k in range(N_CORES):
        sl = slice(k * CPC, (k + 1) * CPC)
        in_maps.append(
            {
                "left": np.ascontiguousarray(left[:, sl]),
                "right": np.ascontiguousarray(right[:, sl]),
            }
        )
    return in_maps


def _assemble(results):
    out = np.empty((B, 2 * C, D, H, W), dtype=np.float32)
    for k in range(N_CORES):
        o = np.asarray(results[k]["out"])  # [B, 2CPC, H, D, W] bf16
        o = np.moveaxis(o, 3, 2)  # view: [B, 2CPC, D, H, W]
        out[:, k * CPC : (k + 1) * CPC] = o[:, :CPC]
        out[:, C + k * CPC : C + (k + 1) * CPC] = o[:, CPC:]
    return out


def run(left_feature, right_feature, max_disp, **spmd_kwargs):
    """Run on hardware; returns (full_output, BassKernelResults)."""
    assert int(max_disp) == D
    left = np.ascontiguousarray(np.asarray(left_feature, dtype=np.float32))
    right = np.ascontiguousarray(np.asarray(right_feature, dtype=np.float32))
    assert left.shape == (B, C, H, W) and right.shape == (B, C, H, W)
    res = run_bass_kernel_spmd(
        _get_nc(), _make_in_maps(left, right), list(range(N_CORES)), **spmd_kwargs
    )
    return _assemble(res.results), res


def kernel(left_feature, right_feature, max_disp):
    out, _ = run(left_feature, right_feature, max_disp)
    return out


# revision 15
# speedup vs baseline: 3.5505x; 3.5505x over previous
"""Concat cost-volume kernel for Trainium2 (8 NeuronCores, SPMD).

Reference semantics (B=2, C=32, H=128, W=240, D=max_disp=48):
  out[b, c,      d, h, w] = left [b, c, h, w]     * (w >= d)   for c in [0, C)
  out[b, C + c,  d, h, w] = right[b, c, h, w - d] * (w >= d)   for c in [0, C)

Pure data movement (~755 MB of f32 output from ~16 MB of input): the kernel
is HBM-write-bandwidth bound. Design (v2, bf16):

* Precision: the correctness gate is rel_err < 2e-2 against the f32
  reference; bf16 rounds inputs once (elementwise rel err <= 2^-8), so the
  whole volume is built and stored as bf16 — HALVING the HBM write traffic —
  and widened back to f32 on the host during unsharding.

* Sharding: channel-parallel. Core k builds the full disparity volume for
  channels [4k, 4k+4) of BOTH halves — all 8 cores run one identical
  program on different channel slices.

* Per-core output layout [B, 2CPC, H, D, W] (disparity moved inward): each
  store descriptor then covers one SBUF partition row's full [D*W] span —
  23 KB contiguous in HBM instead of the 960 B (w >= d staircase) chunks of
  the [D, H, W] layout, which cost ~9% packet overhead (chunk/32+3 beats).
  The host moves D back in front of H during assembly (np view + cast).
  All planes are written in full (zeros included): skipping the w < d wedge
  would fragment descriptors below 512 B and cost more than the ~10% byte
  saving. No reliance on pre-zeroed output buffers.

* Per-core dataflow:
  - left/right slices [2,4,128,240] f32 -> bf16 SBUF via SWDGE cast-loads
    (gpsimd), right into a zero-padded [128, 8*288] tile (48 zero columns in
    front of each plane) so the shifted+masked right plane for disparity d
    is the sliding window cols [48-d : 288-d].
  - staging: per (b, side, c) unit, build [128h, 48d*240w] bf16 in SBUF with
    D-fused DVE ops (broadcast step-0 / sliding step -1 disparity dim).
    bf16 engages the DVE 2x (tensor_tensor) / 4x (copy) perf modes: the
    innermost AP dim is stride-1 two-byte, which is all the mode needs.
    ~70 us/core, fully hidden under the stores.
  - store: one 2.95 MB DMA per unit (16/core), ALL on the single nc.sync
    HWDGE ring, each descriptor a full 23 KB partition row. One ring beats
    two here: with two rings the SDMA engines round-robin between queues at
    packet granularity, interleaving the two write streams and costing ~5%
    of HBM write locality (measured: 133 us 1-ring vs 139 us 2-ring store
    floor). A single ring feeds all 16 SDMA engines, so issue rate is not a
    constraint.

Measured (ABBA-paired loop-delta on HW): stores-only floor ~133 us/core =
355 GB/s = 99% of the 358 GB/s/NC HBM write roofline; full kernel ~139-142
us/core (vs 271 us for the f32 staircase v1, a ~1.9x speedup).
"""

import dataclasses
import sys

import numpy as np

for _p in ("/opt/trn_rl_repo",):
    if _p not in sys.path:
        sys.path.insert(0, _p)

import concourse.bass as bass  # noqa: F401  (kept for interactive debugging)
import concourse.tile as tile
from concourse import bacc, mybir
from concourse.bass_utils import run_bass_kernel_spmd

B, C, H, W = 2, 32, 128, 240
D = 48
N_CORES = 8
CPC = C // N_CORES  # channels per core (per side) = 4
PAD = D  # zero-pad columns = 48
WPAD = W + PAD  # 288
NBC = B * CPC  # input planes per side per core = 8

BEST = dict(gsz=16, bufs=3, dma_engines=("sync",))

_NC_CACHE = {}


def _build_nc(gsz=16, bufs=3, loop_n=None, skip_stores=False,
              skip_compute=False, dma_engines=("sync", "scalar"),
              copy_engine="vector", mul_engine="vector", store_split=1,
              unit_order="bsc", out_dtype="bfloat16", bench_sink=False,
              ups=1, ring_assign="mod"):
    """Build (and compile) the per-core SPMD program.

    Only gsz/bufs (via BEST) affect the production kernel; the other knobs
    exist for benchmarking variants (loop_n wraps the body in a hardware
    For_i for delta timing, skip_* isolate engines, store_split splits each
    unit's store across rings).
    """
    nc = bacc.Bacc("TRN2", target_bir_lowering=False, debug=False)
    f32 = mybir.dt.float32
    bf16 = getattr(mybir.dt, out_dtype)  # on-chip & output dtype
    left_p = nc.declare_dram_parameter("left", [B, CPC, H, W], f32, isOutput=False)
    right_p = nc.declare_dram_parameter("right", [B, CPC, H, W], f32, isOutput=False)
    if bench_sink:
        # Timing-only build: stores hit an Internal DRAM scratch (same
        # instruction stream / HBM behavior), and the ExternalOutput is a
        # tiny token — so per-call wall time is not dominated by fetching
        # the 47 MB/core real output, which would bury the loop delta in
        # host-transfer noise.
        out_p = nc.dram_tensor("sink", [B, 2 * CPC, H, D, W], bf16,
                               kind="Internal")
        tiny_p = nc.declare_dram_parameter("out", [1, 64], f32, isOutput=True)
    else:
        out_p = nc.declare_dram_parameter(
            "out", [B, 2 * CPC, H, D, W], bf16, isOutput=True
        )
        tiny_p = None

    with tile.TileContext(nc) as tc:
        with (
            tc.tile_pool(name="consts", bufs=1) as consts,
            tc.tile_pool(name="stage", bufs=bufs) as stagep,
        ):
            left_t = consts.tile([H, NBC * W], bf16)
            right_t = consts.tile([H, NBC * WPAD], bf16)
            mask_t = consts.tile([H, WPAD], bf16)

            nc.vector.memset(mask_t[:, 0:PAD], 0.0)
            nc.vector.memset(mask_t[:, PAD:WPAD], 1.0)
            nc.gpsimd.memset(right_t[:, :], 0.0)

            if tiny_p is not None:
                tok = consts.tile([1, 64], f32, name="tok")
                nc.vector.memset(tok[:, :], 1.0)
                nc.sync.dma_start(out=tiny_p[:, :], in_=tok[:, :])

            # f32 -> bf16 cast-loads: dtype conversion during DMA is a
            # SWDGE-only feature, so these go on gpsimd.
            nc.gpsimd.dma_start(
                out=left_t[:, :].rearrange("h (k w) -> h k w", w=W),
                in_=left_p[:, :, :, :].rearrange("b c h w -> h (b c) w"),
            )
            nc.gpsimd.dma_start(
                out=right_t[:, :].rearrange("h (k w) -> h k w", w=WPAD)[:, :, PAD:],
                in_=right_p[:, :, :, :].rearrange("b c h w -> h (b c) w"),
            )

            engs = {"sync": nc.sync, "scalar": nc.scalar, "gpsimd": nc.gpsimd,
                    "vector": nc.vector}
            const_st = None
            if skip_compute:
                const_st = consts.tile([H, D * W], bf16, name="const_st")
                nc.vector.memset(const_st[:, :], 0.5)

            def stage_unit(st, side, bc, col0=0):
                """Fill st[:, col0:col0+D*W] with the unit's masked volume.

                One DVE op per group of gsz disparities: the source AP's
                middle dim walks the disparity axis via step 0 (left: same
                plane each d) or step -1 (right / mask: window start col
                48-d slides left as d grows).
                """
                st3d = st[:, col0 : col0 + D * W].rearrange(
                    "h (d w) -> h d w", w=W
                )
                for g0 in range(0, D, gsz):
                    dst = st3d[:, g0 : g0 + gsz, :]
                    if side == 0:
                        lb = left_t[:, bc * W : (bc + 1) * W]
                        mb = mask_t[:, PAD - g0 : PAD - g0 + W]
                        engs[mul_engine].tensor_mul(
                            dst,
                            dataclasses.replace(
                                lb, ap=[lb.ap[0], [0, gsz], [1, W]]
                            ),
                            dataclasses.replace(
                                mb, ap=[mb.ap[0], [-1, gsz], [1, W]]
                            ),
                        )
                    else:
                        rb = right_t[
                            :, bc * WPAD + PAD - g0 : bc * WPAD + PAD - g0 + W
                        ]
                        engs[copy_engine].tensor_copy(
                            dst,
                            dataclasses.replace(
                                rb, ap=[rb.ap[0], [-1, gsz], [1, W]]
                            ),
                        )

            def pick_eng(u):
                n = len(dma_engines)
                if ring_assign == "block":
                    return engs[dma_engines[u * n // (2 * NBC)]]
                return engs[dma_engines[u % n]]

            def store_unit(st, u, b, cc):
                if store_split == 1:
                    eng = pick_eng(u)
                    eng.dma_start(
                        out=out_p[b, cc].rearrange("h d w -> h (d w)"),
                        in_=st[:, :],
                    )
                else:
                    step = D // store_split
                    for si in range(store_split):
                        eng = engs[
                            dma_engines[(u * store_split + si) % len(dma_engines)]
                        ]
                        eng.dma_start(
                            out=out_p[b, cc, :, si * step : (si + 1) * step, :]
                            .rearrange("h d w -> h (d w)"),
                            in_=st[:, si * step * W : (si + 1) * step * W],
                        )

            def store_group(st, g, b, cc0):
                # One DMA covering `ups` channel-consecutive units: dest AP
                # [h][cc: ups][d*w contig] (3 dims, same 23 KB descriptors,
                # 1/ups the DMA count).
                eng = engs[dma_engines[g % len(dma_engines)]]
                eng.dma_start(
                    out=out_p[b, cc0 : cc0 + ups].rearrange(
                        "k h d w -> h k (d w)"
                    ),
                    in_=st[:, :].rearrange("h (k x) -> h k x", k=ups),
                )

            def body():
                if ups > 1:
                    # Groups of `ups` units sharing (b, side), consecutive c.
                    assert CPC % ups == 0
                    for g in range(2 * NBC // ups):
                        b = g // (2 * CPC // ups)
                        side = (g % (2 * CPC // ups)) // (CPC // ups)
                        c0 = (g % (CPC // ups)) * ups
                        st = stagep.tile([H, ups * D * W], bf16, tag="st",
                                         name="st")
                        for j in range(ups):
                            stage_unit(st, side, b * CPC + c0 + j,
                                       col0=j * D * W)
                        if not skip_stores:
                            store_group(st, g, b, side * CPC + c0)
                    return
                if unit_order == "bsc":
                    units = range(2 * NBC)
                else:  # interleave sides: l r l r ...
                    units = [
                        (i // 2) + (i % 2) * CPC + (i // (2 * CPC)) * 2 * CPC
                        for i in range(2 * NBC)
                    ]
                for u in units:
                    b = u // (2 * CPC)
                    side = (u % (2 * CPC)) // CPC
                    c = u % CPC
                    bc = b * CPC + c
                    if skip_compute:
                        st = const_st
                    else:
                        st = stagep.tile([H, D * W], bf16, tag="st", name="st")
                        stage_unit(st, side, bc)
                    if not skip_stores:
                        store_unit(st, u, b, side * CPC + c)

            if loop_n is not None:
                with tc.For_i(0, loop_n, 1):
                    body()
            else:
                body()
    nc.compile()
    return nc


def _get_nc():
    if "nc" not in _NC_CACHE:
        _NC_CACHE["nc"] = _build_nc(**BEST)
    return _NC_CACHE["nc"]


def _make_in_maps(left, right):
    in_maps = []
    for k in range(N_CORES):
        sl = slice(k * CPC, (k + 1) * CPC)
        in_maps.append(
            {
                "left": np.ascontiguousarray(left[:, sl]),
                "right": np.ascontiguousarray(right[:, sl]),
            }
        )
    return in_maps


def _assemble(results):
    out = np.empty((B, 2 * C, D, H, W), dtype=np.float32)
    for k in range(N_CORES):
        o = np.asarray(results[k]["out"])  # [B, 2CPC, H, D, W] bf16
        o = np.moveaxis(o, 3, 2)  # view: [B, 2CPC, D, H, W]
        out[:, k * CPC : (k + 1) * CPC] = o[:, :CPC]
        out[:, C + k * CPC : C + (k + 1) * CPC] = o[:, CPC:]
    return out


def run(left_feature, right_feature, max_disp, **spmd_kwargs):
    """Run on hardware; returns (full_output, BassKernelResults)."""
    assert int(max_disp) == D
    left = np.ascontiguousarray(np.asarray(left_feature, dtype=np.float32))
    right = np.ascontiguousarray(np.asarray(right_feature, dtype=np.float32))
    assert left.shape == (B, C, H, W) and right.shape == (B, C, H, W)
    res = run_bass_kernel_spmd(
        _get_nc(), _make_in_maps(left, right), list(range(N_CORES)), **spmd_kwargs
    )
    return _assemble(res.results), res


def kernel(left_feature, right_feature, max_disp):
    out, _ = run(left_feature, right_feature, max_disp)
    return out
